# revision 19
# baseline (speedup 1.0000x reference)
"""Trainium2 Bass kernel for the CrossAttention (linear-attention style) module.

Math (per batch b, head h, stream A in {x, rgb}):
    K = A @ Wk^T, V = A @ Wv^T
    ctx_A = softmax(scale * K^T V, axis=rows)        # [32, 32] per head
    out_other = Q_other @ blockdiag(ctx_A)           # cross: out_rgb uses ctx_x

Key identity: K^T V = Wk (A^T A) Wv^T, so the big input only feeds the Gram
matrix G = A^T A ([256, 256] per stream) plus the Q-side matmul.

Sharding: 8 cores = 4 batches x 2 streams. Core (b, s) owns stream s of
batch b: it computes G_own -> ctx_own on-chip, then reads the PARTNER
stream's data to form Q^T tiles and the partner-stream output
out = Q_partner @ blockdiag(ctx_own). No cross-core communication.

The bulk data path runs in bf16 (host-side casts); matmuls accumulate in
f32 PSUM. The tiny ctx chain (G, W, TK, softmax) stays f32 — logits are
~+-50 and bf16's 2^-9 relative error there costs ~1.4e-2 output error.
Per-core HBM traffic: 16 MiB read + 8 MiB written, all with
4 KiB-contiguous per-partition DMA runs via the (p o) c tiling.

Phases per core:
  1) stream own[b]: Gram accumulation in PSUM across all 128 tiles
  2) ctx: TK = G Wk^T (4 matmuls), per-head logits = Wv_h^T TK_h (8
     head-pair matmuls, diagonal blocks extracted + packed 4 heads per
     partition block), batched softmax over the free dim, 32x32 DVE
     transposes into the blockdiag bf16 rhs tiles
  3) stream partner[b]: per tile PE-transpose Q -> qT, out matmuls against
     the blockdiag ctx halves (block-aware: 128-col streams), cast + DMA out
"""

import sys

if "/opt/trn_rl_repo" not in sys.path:
    sys.path.insert(0, "/opt/trn_rl_repo")

import numpy as np
import ml_dtypes

import concourse.bass as bass
import concourse.mybir as mybir
import concourse.tile as tile
from concourse import bacc
from concourse.bass import ds, ts
from concourse.bass_utils import run_bass_kernel_spmd

P = 128
C = 256
HD = 32
H = 8
SCALE = HD ** -0.5
F32 = mybir.dt.float32
BF16 = mybir.dt.bfloat16

B_FULL = 4
N_FULL = 16384
N_TILES = N_FULL // P  # 128
T_CHUNK = 8  # tiles per chunk
N_CHUNKS = N_TILES // T_CHUNK  # 16


def build_module(num_devices=8):
    nc = bacc.Bacc(
        "TRN2",
        target_bir_lowering=False,
        debug=False,
        enable_asserts=False,
        num_devices=num_devices,
    )
    a_own = nc.dram_tensor("a_own", [N_FULL, C], BF16, kind="ExternalInput").ap()
    a_par = nc.dram_tensor("a_par", [N_FULL, C], BF16, kind="ExternalInput").ap()
    wkT = nc.dram_tensor("wkT", [C, C], F32, kind="ExternalInput").ap()
    wvT = nc.dram_tensor("wvT", [C, C], F32, kind="ExternalInput").ap()
    ident = nc.dram_tensor("ident", [P, P], BF16, kind="ExternalInput").ap()
    o = nc.dram_tensor("o", [N_FULL, C], BF16, kind="ExternalOutput").ap()

    with tile.TileContext(nc) as tc:
        _build_kernel(tc, a_own, a_par, wkT, wvT, ident, o)
    nc.compile()
    return nc


def _build_kernel(tc, a_own, a_par, wkT_d, wvT_d, ident_d, o):
    nc = tc.nc
    ao_t = a_own.rearrange("(p o) c -> p o c", p=P)  # [128, 128, 256]
    ap_t = a_par.rearrange("(p o) c -> p o c", p=P)
    o_t = o.rearrange("(p o) c -> p o c", p=P)
    wk_t = wkT_d.rearrange("(i p) j -> p i j", p=P)  # [128, 2, 256]
    wv_t = wvT_d.rearrange("(i p) j -> p i j", p=P)

    with (
        tc.tile_pool(name="persist", bufs=1) as persist,
        tc.tile_pool(name="chunks_o", bufs=4) as chunks_o,
        tc.tile_pool(name="chunks_p", bufs=4) as chunks_p,
        tc.tile_pool(name="qtr", bufs=4) as qtr_pool,
        tc.tile_pool(name="outs", bufs=4) as outs,
        tc.tile_pool(name="small", bufs=2) as small,
        tc.tile_pool(name="psum_g", bufs=1, space="PSUM") as psum_g,
        tc.tile_pool(name="psum_t", bufs=2, space="PSUM") as psum_t,
        tc.tile_pool(name="psum_o", bufs=2, space="PSUM") as psum_o,
        tc.tile_pool(name="psum_s", bufs=1, space="PSUM") as psum_s,
    ):
        # ---- persistent state ----
        # the ctx chain (G, W, TK) stays f32: softmax logits are ~+-50 and
        # bf16's 2^-9 relative error there costs ~1.4e-2 output error
        w_k = persist.tile([P, 2, C], F32, tag="w_k")
        w_v = persist.tile([P, 2, C], F32, tag="w_v")
        ident = persist.tile([P, P], BF16, tag="ident")
        g = persist.tile([P, 2, C], F32, tag="g")  # G rows: [half i][128, 256]
        tk = persist.tile([P, 2, C], F32, tag="tk")  # TK rows: [half ci][128, 256]
        rhs_blk = persist.tile([P, 2, P], BF16, tag="rhs_blk")  # blockdiag ctx

        nc.sync.dma_start(w_k[:], wk_t)
        nc.sync.dma_start(w_v[:], wv_t)
        nc.sync.dma_start(ident[:], ident_d)
        nc.vector.memset(rhs_blk[:].bitcast(mybir.dt.uint16), 0)

        # ---- phase 1: Gram of own stream, accumulated in PSUM ----
        pg0 = psum_g.tile([P, C], F32, tag="pg0", name="pg0")  # G rows 0:128
        pg1 = psum_g.tile([P, C], F32, tag="pg1", name="pg1")  # G rows 128:256
        for ch in range(N_CHUNKS):
            in_sb = chunks_o.tile([P, T_CHUNK, C], BF16, tag="chunk_o")
            nc.sync.dma_start(in_sb[:], ao_t[:, ts(ch, T_CHUNK), :])
            for t in range(T_CHUNK):
                tl = in_sb[:, t, :]
                first = ch == 0 and t == 0
                last = ch == N_CHUNKS - 1 and t == T_CHUNK - 1
                nc.tensor.matmul(pg0[:], tl[:, 0:P], tl, start=first, stop=last)
                nc.tensor.matmul(pg1[:], tl[:, P:C], tl, start=first, stop=last)

        # ---- phase 2: ctx of own stream ----
        nc.vector.tensor_copy(g[:, 0, :], pg0[:])
        nc.vector.tensor_copy(g[:, 1, :], pg1[:])

        # TK[c', j] = sum_c G[c, c'] WkT[c, j]  (rows c' half i)
        tk_ps = psum_s.tile([P, 2, C], F32, tag="tk_ps", name="tk_ps")
        for i in range(2):
            for ci in range(2):
                nc.tensor.matmul(
                    tk_ps[:, i, :],
                    g[:, ci, ts(i, P)],
                    w_k[:, ci, :],
                    start=(ci == 0),
                    stop=(ci == 1),
                )
        nc.vector.tensor_copy(tk[:], tk_ps[:])

        # head-pair logit matmuls: for pair (h, h+1), out [64, 64] has the
        # valid per-head [32,32] blocks M_h^T on its diagonal (PE output
        # base partition must be 0/32/64, so heads can't pack at offset 96).
        # Reuses the tk_ps bank: tk is already copied to SBUF by then.
        for i in range(2):
            for q in range(2):
                h0 = 4 * i + 2 * q
                for ci in range(2):
                    nc.tensor.matmul(
                        tk_ps[ds(2 * HD * q, 2 * HD), i, 0 : 2 * HD],
                        w_v[:, ci, ds(HD * h0, 2 * HD)],
                        tk[:, ci, ds(HD * h0, 2 * HD)],
                        start=(ci == 0),
                        stop=(ci == 1),
                    )
        # extract the valid diagonal blocks into packed [128, 2, 32] layout:
        # row 32k+e of plane i holds M_{4i+k}[.., e]
        pl = small.tile([P, 2, HD], F32, tag="pl")
        for i in range(2):
            for k in range(4):
                s = k % 2
                nc.vector.tensor_copy(
                    pl[ds(HD * k, HD), i, :],
                    tk_ps[ds(HD * k, HD), i, ds(HD * s, HD)],
                )
        # batched softmax over free dim d (scale folded into exp)
        for i in range(2):
            mx = small.tile([P, 1], F32, tag=f"mx{i}")
            nc.vector.tensor_reduce(
                mx[:], pl[:, i, :], axis=mybir.AxisListType.X, op=mybir.AluOpType.max
            )
            nmx = small.tile([P, 1], F32, tag=f"nmx{i}")
            nc.vector.tensor_scalar_mul(nmx[:], mx[:], -SCALE)
            sm = small.tile([P, HD], F32, tag=f"sm{i}")
            ssum = small.tile([P, 1], F32, tag=f"ssum{i}")
            nc.scalar.activation(
                sm[:],
                pl[:, i, :],
                mybir.ActivationFunctionType.Exp,
                bias=nmx[:],
                scale=SCALE,
                accum_out=ssum[:],
            )
            rs = small.tile([P, 1], F32, tag=f"rs{i}")
            nc.vector.reciprocal(rs[:], ssum[:])
            smn = small.tile([P, HD], F32, tag=f"smn{i}")
            nc.vector.tensor_scalar_mul(smn[:], sm[:], rs[:])
            # per-head 32x32 transpose [e,d] -> [d,e] (StreamTranspose can't
            # cast, so transpose f32->f32 then cast into the blockdiag slot)
            nat = small.tile([P, HD], F32, tag=f"nat{i}")
            for k in range(4):
                nc.vector.transpose(
                    nat[ds(HD * k, HD), :],
                    smn[ds(HD * k, HD), :],
                )
                nc.vector.tensor_copy(
                    rhs_blk[ds(HD * k, HD), i, ds(HD * k, HD)],
                    nat[ds(HD * k, HD), :],
                )

        # ---- phase 3: partner stream -> qT tiles -> out matmuls ----
        for ch in range(N_CHUNKS):
            pin = chunks_p.tile([P, T_CHUNK, C], BF16, tag="chunk_p")
            nc.sync.dma_start(pin[:], ap_t[:, ts(ch, T_CHUNK), :])
            qch = qtr_pool.tile([P, T_CHUNK, 2, P], BF16, tag="qch")
            out_sb = outs.tile([P, T_CHUNK, C], BF16, tag="o_stage")
            for t in range(T_CHUNK):
                tl = pin[:, t, :]
                tp = psum_t.tile([P, 2, P], BF16, tag="tp")
                nc.tensor.transpose(tp[:, 0, :], tl[:, 0:P], ident[:])
                nc.tensor.transpose(tp[:, 1, :], tl[:, P:C], ident[:])
                if t % 2 == 0:
                    nc.vector.tensor_copy(qch[:, t, :, :], tp[:])
                else:
                    nc.scalar.copy(qch[:, t, :, :], tp[:])
                po = psum_o.tile([P, C], F32, tag="po")
                nc.tensor.matmul(
                    po[:, 0:P], qch[:, t, 0, :], rhs_blk[:, 0, :], start=True, stop=True
                )
                nc.tensor.matmul(
                    po[:, P:C], qch[:, t, 1, :], rhs_blk[:, 1, :], start=True, stop=True
                )
                if t % 2 == 0:
                    nc.scalar.copy(out_sb[:, t, :], po[:])
                else:
                    nc.vector.tensor_copy(out_sb[:, t, :], po[:])
            nc.sync.dma_start(o_t[:, ts(ch, T_CHUNK), :], out_sb[:])


# ---------------------------------------------------------------------------
# Host-side wrapper
# ---------------------------------------------------------------------------

_NC_CACHE = {}


def _get_module(**kw):
    key = tuple(sorted(kw.items()))
    if key not in _NC_CACHE:
        _NC_CACHE[key] = build_module(**kw)
    return _NC_CACHE[key]


def make_in_maps(rgb, x, Wkv_rgb, Wkv_x, n_cores=8):
    """Per-core input dicts. Core (b, s): own stream s (0=x, 1=rgb) of batch
    b feeds the Gram/ctx; the partner stream feeds Q and the output."""
    bf = ml_dtypes.bfloat16
    x_b = [np.ascontiguousarray(x[b]).astype(bf) for b in range(B_FULL)]
    r_b = [np.ascontiguousarray(rgb[b]).astype(bf) for b in range(B_FULL)]
    wk = {0: np.ascontiguousarray(Wkv_x[0:C].T, dtype=np.float32),
          1: np.ascontiguousarray(Wkv_rgb[0:C].T, dtype=np.float32)}
    wv = {0: np.ascontiguousarray(Wkv_x[C:2 * C].T, dtype=np.float32),
          1: np.ascontiguousarray(Wkv_rgb[C:2 * C].T, dtype=np.float32)}
    eye = np.eye(P, dtype=np.float32).astype(bf)
    in_maps = []
    for core in range(n_cores):
        b, s = divmod(core, 2)
        own = x_b[b] if s == 0 else r_b[b]
        par = r_b[b] if s == 0 else x_b[b]
        in_maps.append(
            {
                "a_own": own,
                "a_par": par,
                "wkT": wk[s],
                "wvT": wv[s],
                "ident": eye,
            }
        )
    return in_maps


def assemble(results):
    out_rgb = np.empty((B_FULL, N_FULL, C), dtype=np.float32)
    out_x = np.empty_like(out_rgb)
    for core, res in enumerate(results):
        b, s = divmod(core, 2)
        # core owns stream s -> produced the OTHER stream's output
        dst = out_rgb if s == 0 else out_x
        dst[b] = res["o"].astype(np.float32)
    return out_rgb, out_x


def kernel(rgb, x, Wkv_rgb, Wkv_x, num_heads):
    rgb = np.asarray(rgb, dtype=np.float32)
    x = np.asarray(x, dtype=np.float32)
    Wkv_rgb = np.asarray(Wkv_rgb, dtype=np.float32)
    Wkv_x = np.asarray(Wkv_x, dtype=np.float32)
    assert int(num_heads) == H
    assert rgb.shape == (B_FULL, N_FULL, C) and x.shape == (B_FULL, N_FULL, C)

    nc = _get_module()
    in_maps = make_in_maps(rgb, x, Wkv_rgb, Wkv_x)
    res = run_bass_kernel_spmd(nc, in_maps, core_ids=list(range(8)))
    return assemble(res.results)
